# revision 7
# baseline (speedup 1.0000x reference)
"""Causal single-head dot-product attention + output projection on 8 TRN2 cores.

Problem (hardcoded): B=4, S=2048, H=16, D=64 -> E=1024 (heads flattened).
  q = query.reshape(B,S,E) * E**-0.5
  scores = q @ k^T  (causal mask)  -> softmax -> @ v -> @ out_w.T + out_b

Sharding: core c = 2*b + p  (batch b, parity p) owns query rows {p, p+2, ...}
of batch b (1024 rows).  Row r attends keys <= r, so local q-tile t
(256 local rows = global rows ~[512t, 512t+512)) needs only keys < 512(t+1):
per-core causal work is identical across cores -> one SPMD program.

On-chip layout: scores are computed transposed, S^T[k, q] = (K^T)^T-free
matmul with lhsT=K^T tiles, rhs=Q^T tiles.  exp(S^T) is then directly the
rhs for O^T[e, q] = V^T-free matmul (lhsT=V tiles), and O^T tiles are
directly the lhsT for Y[q, eo] = O @ W^T.  No on-chip transposes anywhere.
Softmax is computed without max-subtraction (scores ~ N(0,1) after the
1/32 scale, exp cannot overflow); row sums l[q] come from a ones-vector
matmul on PE; normalization is folded into the final projection:
   Y_psum = l[q]*b[eo] + O_unnorm @ W^T ;  Y = Y_psum / l[q]  = O@W^T + b.

All matmuls run in float32r (full-rate fp32-storage mode, ~1e-4 rel err).
"""

import numpy as np

import concourse.bass as bass
import concourse.tile as tile
from concourse import bacc, mybir
from concourse.bass_utils import run_bass_kernel_spmd

B, S, H, D = 4, 2048, 16, 64
E = H * D  # 1024
P = 128
NT = 4  # q tiles per core
QW = 256  # q tile width (local rows)
ESUB = E // P  # 8
NCORES = 8
F32 = mybir.dt.float32
F32R = mybir.dt.float32r
NEG = -1.0e30


def _build_program(causal: bool, reps: int = 1):
    nc = bacc.Bacc("TRN2", target_bir_lowering=False, debug=False)

    # DRAM parameters (per-core data).  Block-major layouts so every DMA is
    # contiguous.
    qt_d = nc.dram_tensor("qt", [NT, P, ESUB, QW], F32R, kind="ExternalInput").ap()
    kt_d = nc.dram_tensor("kt", [4, P, ESUB, 512], F32R, kind="ExternalInput").ap()
    v_d = nc.dram_tensor("v", [4, P, 4, E], F32R, kind="ExternalInput").ap()
    wt_d = nc.dram_tensor("wt", [P, ESUB, E], F32R, kind="ExternalInput").ap()
    masks_d = nc.dram_tensor("masks", [P, 4, QW], F32, kind="ExternalInput").ap()
    ones_d = nc.dram_tensor("ones", [P, 1], F32R, kind="ExternalInput").ap()
    y_d = nc.dram_tensor("y", [NT * QW, E], F32, kind="ExternalOutput").ap()
    lsum_d = nc.dram_tensor("lsum", [NT, QW], F32, kind="ExternalOutput").ap()

    with tile.TileContext(nc) as tc:
        with (
            tc.tile_pool(name="const", bufs=1) as const,
            tc.tile_pool(name="kvstream", bufs=2) as kvstream,
            tc.tile_pool(name="qpool", bufs=2) as qpool,
            tc.tile_pool(name="ptpool", bufs=1) as ptpool,
            tc.tile_pool(name="otpool", bufs=1) as otpool,
            tc.tile_pool(name="ypool", bufs=2) as ypool,
            tc.tile_pool(name="small", bufs=4) as small,
            tc.tile_pool(name="ps_ot", bufs=1, space="PSUM") as ps_ot,
            tc.tile_pool(name="ps_work", bufs=3, space="PSUM") as ps_work,
            tc.tile_pool(name="ps_sums", bufs=1, space="PSUM") as ps_sums,
        ):
            # ---- resident constants ----
            kt01 = const.tile([P, 2, ESUB, 512], F32R)  # key blocks 0,1
            v01 = const.tile([P, 2, 4, E], F32R)  # value blocks 0,1
            for kb in range(2):
                nc.sync.dma_start(kt01[:, kb], kt_d[kb])
                nc.sync.dma_start(v01[:, kb], v_d[kb])
            wt_sb = const.tile([P, ESUB, E], F32R)
            nc.sync.dma_start(wt_sb, wt_d[:])
            masks_sb = const.tile([P, 4, QW], F32)
            nc.sync.dma_start(masks_sb, masks_d[:])
            ones_col = const.tile([P, 1], F32R)
            nc.sync.dma_start(ones_col, ones_d[:])

            for _rep in range(reps):
                for t in range(NT):
                    nkb = (t + 1) if causal else 4
                    nksub = 4 * nkb

                    qt_t = qpool.tile([P, ESUB, QW], F32R, tag="qt")
                    nc.sync.dma_start(qt_t, qt_d[t])

                    pt_t = ptpool.tile([P, 4 * NT, QW], F32R, tag="pt")
                    sums_ps = ps_sums.tile([1, QW], F32, tag="sums")

                    # ---- phase A: S^T = K^T-blocks x Q^T, mask, exp, sums ----
                    for kb in range(nkb):
                        if kb < 2:
                            kt_blk = kt01[:, kb]
                        else:
                            kt_blk = kvstream.tile([P, ESUB, 512], F32R, tag="kv")
                            nc.sync.dma_start(kt_blk, kt_d[kb])
                        for pair in range(2):
                            st = ps_work.tile([P, 2, QW], F32, tag="work")
                            for j in range(2):
                                sloc = 2 * pair + j
                                for e in range(ESUB):
                                    nc.tensor.matmul(
                                        st[:, j],
                                        kt_blk[:, e, 128 * sloc : 128 * (sloc + 1)],
                                        qt_t[:, e, :],
                                        start=(e == 0),
                                        stop=(e == ESUB - 1),
                                    )
                            if causal and kb == t:
                                nc.vector.tensor_add(
                                    st[:], st[:], masks_sb[:, 2 * pair : 2 * pair + 2, :]
                                )
                            ks0 = 4 * kb + 2 * pair
                            nc.scalar.activation(
                                out=pt_t[:, ks0 : ks0 + 2, :],
                                in_=st[:],
                                func=mybir.ActivationFunctionType.Exp,
                                scale=float(E) ** -0.5,
                            )
                            for j in range(2):
                                ks = ks0 + j
                                nc.tensor.matmul(
                                    sums_ps[:],
                                    ones_col[:],
                                    pt_t[:, ks, :],
                                    start=(ks == 0),
                                    stop=(ks == nksub - 1),
                                )

                    # ---- phase B: O^T[e, q] accumulate over key blocks ----
                    ot_ps = [
                        ps_ot.tile([P, 2, QW], F32, tag=f"ot{i}", name=f"ot{i}")
                        for i in range(4)
                    ]
                    v_blks = []
                    for kb in range(nkb):
                        if kb < 2:
                            v_blks.append(v01[:, kb])
                        else:
                            v_blk = kvstream.tile([P, 4, E], F32R, tag="kv")
                            nc.sync.dma_start(v_blk, v_d[kb])
                            v_blks.append(v_blk)
                    # Within one PSUM bank (one ot tile) the two half-bank
                    # accumulation groups must run sequentially: start=True
                    # clears has_written for the whole bank.
                    for epair in range(4):
                        for j in range(2):
                            e = 2 * epair + j
                            for kb in range(nkb):
                                for sloc in range(4):
                                    ks = 4 * kb + sloc
                                    nc.tensor.matmul(
                                        ot_ps[epair][:, j],
                                        v_blks[kb][:, sloc, 128 * e : 128 * (e + 1)],
                                        pt_t[:, ks, :],
                                        start=(ks == 0),
                                        stop=(ks == nksub - 1),
                                    )

                    ot_sb = otpool.tile([P, ESUB, QW], F32R, tag="ot_sb")
                    for epair in range(4):
                        nc.vector.tensor_copy(
                            ot_sb[:, 2 * epair : 2 * epair + 2, :], ot_ps[epair][:]
                        )

                    # row sums -> DRAM (normalization + bias happen on host)
                    sums_sb = small.tile([1, QW], F32, tag="sums_sb")
                    nc.vector.tensor_copy(sums_sb[:], sums_ps[:])
                    nc.sync.dma_start(lsum_d[t : t + 1, :], sums_sb[:])

                    # ---- phase C: Y_un[q, eo] = O_un @ W^T ----
                    for qs in range(2):
                        y_sb = ypool.tile([P, 2, 512], F32, tag="y")
                        for eh in range(2):
                            yp = ps_work.tile([P, 512], F32, tag="work")
                            for e in range(ESUB):
                                nc.tensor.matmul(
                                    yp,
                                    ot_sb[:, e, 128 * qs : 128 * (qs + 1)],
                                    wt_sb[:, e, 512 * eh : 512 * (eh + 1)],
                                    start=(e == 0),
                                    stop=(e == ESUB - 1),
                                )
                            nc.vector.tensor_copy(y_sb[:, eh], yp)
                        nc.sync.dma_start(
                            y_d[QW * t + 128 * qs : QW * t + 128 * (qs + 1), :],
                            y_sb[:],
                        )
    nc.compile()
    return nc


_PROGRAM_CACHE: dict = {}


def _get_program(causal: bool, reps: int = 1):
    key = (causal, reps)
    if key not in _PROGRAM_CACHE:
        _PROGRAM_CACHE[key] = _build_program(causal, reps)
    return _PROGRAM_CACHE[key]


def _sb_layout_T(x2d: np.ndarray, nsub: int) -> np.ndarray:
    """[K, N] -> SBUF contraction layout [128, nsub, N] with K = nsub*128."""
    return np.ascontiguousarray(x2d.reshape(nsub, P, -1).transpose(1, 0, 2))


def _make_in_maps(query, key, value, out_w, causal_parity_masks):
    q3 = query.reshape(B, S, E)
    k3 = key.reshape(B, S, E)
    v3 = value.reshape(B, S, E)

    wt = _sb_layout_T(np.ascontiguousarray(out_w.T), ESUB)  # [128, 8, 1024]

    in_maps = []
    for c in range(NCORES):
        b, p = divmod(c, 2)
        # Q^T for this core's interleaved rows, tile-major.
        qc = np.ascontiguousarray(q3[b, p::2].T)  # [E, 1024]
        qt_sb = _sb_layout_T(qc, ESUB)  # [128, 8, 1024]
        qt = np.ascontiguousarray(
            qt_sb.reshape(P, ESUB, NT, QW).transpose(2, 0, 1, 3)
        )  # [NT, 128, 8, 256]
        # K^T block-major: [4, 128, 8, 512]
        ktc = _sb_layout_T(np.ascontiguousarray(k3[b].T), ESUB)  # [128, 8, 2048]
        kt = np.ascontiguousarray(ktc.reshape(P, ESUB, 4, 512).transpose(2, 0, 1, 3))
        # V block-major: [4, 128, 4, 1024] (partition = key-row % 128)
        vc = v3[b].reshape(4, 4, P, E).transpose(0, 2, 1, 3)
        vc = np.ascontiguousarray(vc)
        in_maps.append(
            {
                "qt": qt.astype(np.float32),
                "kt": kt.astype(np.float32),
                "v": vc.astype(np.float32),
                "wt": wt.astype(np.float32),
                "masks": causal_parity_masks[p],
                "ones": np.ones((P, 1), dtype=np.float32),
                "y": None,  # output
            }
        )
    for m in in_maps:
        del m["y"]
    return in_maps


def _parity_masks():
    """masks[p][kk, s, i] = NEG where key (128*s + kk) of the diagonal band
    is masked for local row i of parity p (global row = 2*i + p mod 512)."""
    out = []
    kk = np.arange(P)[:, None, None]
    s = np.arange(4)[None, :, None]
    i = np.arange(QW)[None, None, :]
    for p in range(2):
        m = np.where(128 * s + kk > 2 * i + p, np.float32(NEG), np.float32(0.0))
        out.append(np.ascontiguousarray(m.astype(np.float32)))
    return out


def _numpy_fallback(query, key, value, attn_mask, out_w, out_b):
    q = query.reshape(B, S, E).astype(np.float64) * (float(E) ** -0.5)
    k = key.reshape(B, S, E).astype(np.float64)
    v = value.reshape(B, S, E).astype(np.float64)
    scores = np.einsum("bqe,bke->bqk", q, k)
    scores = np.where(attn_mask[None, :, :] == 0, -np.inf, scores)
    scores -= scores.max(axis=-1, keepdims=True)
    probs = np.exp(scores)
    probs /= probs.sum(axis=-1, keepdims=True)
    attn = np.einsum("bqk,bke->bqe", probs, v)
    return (attn @ out_w.T.astype(np.float64) + out_b.astype(np.float64)).astype(
        np.float32
    )


def kernel(query, key, value, qkv_proj, attn_mask, out_w, out_b):
    del qkv_proj
    mask = np.asarray(attn_mask)
    is_causal = bool(
        np.array_equal(mask, np.tril(np.ones((S, S), dtype=mask.dtype)))
    )
    is_full = bool((mask != 0).all())
    if not (is_causal or is_full):
        return _numpy_fallback(query, key, value, mask, out_w, out_b)

    query = np.asarray(query, dtype=np.float32)
    key = np.asarray(key, dtype=np.float32)
    value = np.asarray(value, dtype=np.float32)
    out_w = np.asarray(out_w, dtype=np.float32)
    out_b = np.asarray(out_b, dtype=np.float32)

    nc = _get_program(causal=is_causal)
    in_maps = _make_in_maps(query, key, value, out_w, _parity_masks())
    res = run_bass_kernel_spmd(nc, in_maps, list(range(NCORES)))

    out = np.empty((B, S, E), dtype=np.float32)
    for c in range(NCORES):
        b, p = divmod(c, 2)
        y_un = res.results[c]["y"]
        lsum = res.results[c]["lsum"].reshape(NT * QW, 1)
        out[b, p::2, :] = y_un / lsum + out_b[None, :]
    return out


if __name__ == "__main__":
    rng = np.random.default_rng(0)
    q = rng.standard_normal((B, S, H, D), dtype=np.float32)
    k = rng.standard_normal((B, S, H, D), dtype=np.float32)
    v = rng.standard_normal((B, S, H, D), dtype=np.float32)
    w = rng.standard_normal((E, E), dtype=np.float32) * (1.0 / 32)
    bb = rng.standard_normal((E,), dtype=np.float32) * (1.0 / 32)
    m = np.tril(np.ones((S, S), dtype=np.int32))
    y = kernel(
        query=q, key=k, value=v, qkv_proj=np.zeros(1, np.float32),
        attn_mask=m, out_w=w, out_b=bb,
    )
    ref = _numpy_fallback(q, k, v, m, w, bb)
    err = np.abs(y - ref)
    rel = err.max() / np.abs(ref).max()
    print("quick self-check: absmax rel err =", rel)


# revision 8
# speedup vs baseline: 23.1881x; 23.1881x over previous
"""Causal single-head dot-product attention + output projection on 8 TRN2 cores.

Problem (hardcoded): B=4, S=2048, H=16, D=64 -> E=1024 (heads flattened).
  q = query.reshape(B,S,E) * E**-0.5
  scores = q @ k^T  (causal mask)  -> softmax -> @ v -> @ out_w.T + out_b

Sharding: core c = 2*b + p  (batch b, parity p) owns query rows {p, p+2, ...}
of batch b (1024 rows).  Row r attends keys <= r, so local q-tile t
(256 local rows = global rows ~[512t, 512t+512)) needs only keys < 512(t+1):
per-core causal work is identical across cores -> one SPMD program.

On-chip layout: scores are computed transposed, S^T[k, q] = (K^T)^T-free
matmul with lhsT=K^T tiles, rhs=Q^T tiles.  exp(S^T) is then directly the
rhs for O^T[e, q] = V^T-free matmul (lhsT=V tiles), and O^T tiles are
directly the lhsT for Y[q, eo] = O @ W^T.  No on-chip transposes anywhere.
Softmax is computed without max-subtraction (scores ~ N(0,1) after the
1/32 scale, exp cannot overflow); row sums l[q] come from a ones-vector
matmul on PE; normalization is folded into the final projection:
   Y_psum = l[q]*b[eo] + O_unnorm @ W^T ;  Y = Y_psum / l[q]  = O@W^T + b.

All matmuls run in float32r (full-rate fp32-storage mode, ~1e-4 rel err).
"""

import numpy as np

import concourse.bass as bass
import concourse.tile as tile
from concourse import bacc, mybir
from concourse.bass_utils import run_bass_kernel_spmd

B, S, H, D = 4, 2048, 16, 64
E = H * D  # 1024
P = 128
NT = 4  # q tiles per core
QW = 256  # q tile width (local rows)
ESUB = E // P  # 8
NCORES = 8
F32 = mybir.dt.float32
F32R = mybir.dt.float32r
NEG = -1.0e30


def _build_program(causal: bool, reps: int = 1):
    nc = bacc.Bacc("TRN2", target_bir_lowering=False, debug=False)

    # DRAM parameters (per-core data).  Block-major layouts so every DMA is
    # contiguous.
    qt_d = nc.dram_tensor("qt", [NT, P, ESUB, QW], F32R, kind="ExternalInput").ap()
    kt_d = nc.dram_tensor("kt", [4, P, ESUB, 512], F32R, kind="ExternalInput").ap()
    v_d = nc.dram_tensor("v", [4, P, 4, E], F32R, kind="ExternalInput").ap()
    wt_d = nc.dram_tensor("wt", [P, ESUB, E], F32R, kind="ExternalInput").ap()
    masks_d = nc.dram_tensor("masks", [P, 4, QW], F32, kind="ExternalInput").ap()
    ones_d = nc.dram_tensor("ones", [P, 1], F32R, kind="ExternalInput").ap()
    y_d = nc.dram_tensor("y", [NT * QW, E], F32, kind="ExternalOutput").ap()
    lsum_d = nc.dram_tensor("lsum", [NT, QW], F32, kind="ExternalOutput").ap()

    with tile.TileContext(nc) as tc:
        with (
            tc.tile_pool(name="const", bufs=1) as const,
            tc.tile_pool(name="kvstream", bufs=3) as kvstream,
            tc.tile_pool(name="qpool", bufs=2) as qpool,
            tc.tile_pool(name="ptpool", bufs=1) as ptpool,
            tc.tile_pool(name="otpool", bufs=1) as otpool,
            tc.tile_pool(name="ypool", bufs=2) as ypool,
            tc.tile_pool(name="small", bufs=4) as small,
            tc.tile_pool(name="ps_ot", bufs=1, space="PSUM") as ps_ot,
            tc.tile_pool(name="ps_work", bufs=3, space="PSUM") as ps_work,
            tc.tile_pool(name="ps_sums", bufs=1, space="PSUM") as ps_sums,
        ):
            # ---- resident constants ----
            kt01 = const.tile([P, 2, ESUB, 512], F32R)  # key blocks 0,1
            v01 = const.tile([P, 2, 4, E], F32R)  # value blocks 0,1
            for kb in range(2):
                nc.sync.dma_start(kt01[:, kb], kt_d[kb])
                nc.sync.dma_start(v01[:, kb], v_d[kb])
            wt_sb = const.tile([P, ESUB, E], F32R)
            nc.sync.dma_start(wt_sb, wt_d[:])
            masks_sb = const.tile([P, 4, QW], F32)
            nc.sync.dma_start(masks_sb, masks_d[:])
            ones_col = const.tile([P, 1], F32R)
            nc.sync.dma_start(ones_col, ones_d[:])

            for _rep in range(reps):
                for t in range(NT):
                    nkb = (t + 1) if causal else 4
                    nksub = 4 * nkb

                    qt_t = qpool.tile([P, ESUB, QW], F32R, tag="qt")
                    nc.sync.dma_start(qt_t, qt_d[t])

                    pt_t = ptpool.tile([P, 4 * NT, QW], F32R, tag="pt")
                    sums_ps = ps_sums.tile([1, QW], F32, tag="sums")

                    # ---- phase A: S^T = K^T-blocks x Q^T, mask, exp, sums ----
                    for kb in range(nkb):
                        if kb < 2:
                            kt_blk = kt01[:, kb]
                        else:
                            kt_blk = kvstream.tile([P, ESUB, 512], F32R, tag="kv")
                            nc.sync.dma_start(kt_blk, kt_d[kb])
                        for pair in range(2):
                            st = ps_work.tile([P, 2, QW], F32, tag="work")
                            for j in range(2):
                                sloc = 2 * pair + j
                                for e in range(ESUB):
                                    nc.tensor.matmul(
                                        st[:, j],
                                        kt_blk[:, e, 128 * sloc : 128 * (sloc + 1)],
                                        qt_t[:, e, :],
                                        start=(e == 0),
                                        stop=(e == ESUB - 1),
                                    )
                            if causal and kb == t:
                                nc.vector.tensor_add(
                                    st[:], st[:], masks_sb[:, 2 * pair : 2 * pair + 2, :]
                                )
                            ks0 = 4 * kb + 2 * pair
                            nc.scalar.activation(
                                out=pt_t[:, ks0 : ks0 + 2, :],
                                in_=st[:],
                                func=mybir.ActivationFunctionType.Exp,
                                scale=float(E) ** -0.5,
                            )
                            for j in range(2):
                                ks = ks0 + j
                                nc.tensor.matmul(
                                    sums_ps[:],
                                    ones_col[:],
                                    pt_t[:, ks, :],
                                    start=(ks == 0),
                                    stop=(ks == nksub - 1),
                                )

                    # ---- phase B: O^T[e, q] accumulate over key blocks ----
                    ot_ps = [
                        ps_ot.tile([P, 2, QW], F32, tag=f"ot{i}", name=f"ot{i}")
                        for i in range(4)
                    ]
                    v_blks = []
                    for kb in range(nkb):
                        if kb < 2:
                            v_blks.append(v01[:, kb])
                        else:
                            v_blk = kvstream.tile([P, 4, E], F32R, tag="kv")
                            nc.sync.dma_start(v_blk, v_d[kb])
                            v_blks.append(v_blk)
                    # Within one PSUM bank (one ot tile) the two half-bank
                    # accumulation groups must run sequentially: start=True
                    # clears has_written for the whole bank.
                    for epair in range(4):
                        for j in range(2):
                            e = 2 * epair + j
                            for kb in range(nkb):
                                for sloc in range(4):
                                    ks = 4 * kb + sloc
                                    nc.tensor.matmul(
                                        ot_ps[epair][:, j],
                                        v_blks[kb][:, sloc, 128 * e : 128 * (e + 1)],
                                        pt_t[:, ks, :],
                                        start=(ks == 0),
                                        stop=(ks == nksub - 1),
                                    )

                    ot_sb = otpool.tile([P, ESUB, QW], F32R, tag="ot_sb")
                    for epair in range(4):
                        nc.vector.tensor_copy(
                            ot_sb[:, 2 * epair : 2 * epair + 2, :], ot_ps[epair][:]
                        )

                    # row sums -> DRAM (normalization + bias happen on host)
                    sums_sb = small.tile([1, QW], F32, tag="sums_sb")
                    nc.vector.tensor_copy(sums_sb[:], sums_ps[:])
                    nc.sync.dma_start(lsum_d[t : t + 1, :], sums_sb[:])

                    # ---- phase C: Y_un[q, eo] = O_un @ W^T ----
                    for qs in range(2):
                        y_sb = ypool.tile([P, 2, 512], F32, tag="y")
                        for eh in range(2):
                            yp = ps_work.tile([P, 512], F32, tag="work")
                            for e in range(ESUB):
                                nc.tensor.matmul(
                                    yp,
                                    ot_sb[:, e, 128 * qs : 128 * (qs + 1)],
                                    wt_sb[:, e, 512 * eh : 512 * (eh + 1)],
                                    start=(e == 0),
                                    stop=(e == ESUB - 1),
                                )
                            nc.vector.tensor_copy(y_sb[:, eh], yp)
                        nc.sync.dma_start(
                            y_d[QW * t + 128 * qs : QW * t + 128 * (qs + 1), :],
                            y_sb[:],
                        )
    nc.compile()
    return nc


_PROGRAM_CACHE: dict = {}


def _get_program(causal: bool, reps: int = 1):
    key = (causal, reps)
    if key not in _PROGRAM_CACHE:
        _PROGRAM_CACHE[key] = _build_program(causal, reps)
    return _PROGRAM_CACHE[key]


def _sb_layout_T(x2d: np.ndarray, nsub: int) -> np.ndarray:
    """[K, N] -> SBUF contraction layout [128, nsub, N] with K = nsub*128."""
    return np.ascontiguousarray(x2d.reshape(nsub, P, -1).transpose(1, 0, 2))


def _make_in_maps(query, key, value, out_w, causal_parity_masks):
    q3 = query.reshape(B, S, E)
    k3 = key.reshape(B, S, E)
    v3 = value.reshape(B, S, E)

    wt = _sb_layout_T(np.ascontiguousarray(out_w.T), ESUB)  # [128, 8, 1024]

    in_maps = []
    for c in range(NCORES):
        b, p = divmod(c, 2)
        # Q^T for this core's interleaved rows, tile-major.
        qc = np.ascontiguousarray(q3[b, p::2].T)  # [E, 1024]
        qt_sb = _sb_layout_T(qc, ESUB)  # [128, 8, 1024]
        qt = np.ascontiguousarray(
            qt_sb.reshape(P, ESUB, NT, QW).transpose(2, 0, 1, 3)
        )  # [NT, 128, 8, 256]
        # K^T block-major: [4, 128, 8, 512]
        ktc = _sb_layout_T(np.ascontiguousarray(k3[b].T), ESUB)  # [128, 8, 2048]
        kt = np.ascontiguousarray(ktc.reshape(P, ESUB, 4, 512).transpose(2, 0, 1, 3))
        # V block-major: [4, 128, 4, 1024] (partition = key-row % 128)
        vc = v3[b].reshape(4, 4, P, E).transpose(0, 2, 1, 3)
        vc = np.ascontiguousarray(vc)
        in_maps.append(
            {
                "qt": qt.astype(np.float32),
                "kt": kt.astype(np.float32),
                "v": vc.astype(np.float32),
                "wt": wt.astype(np.float32),
                "masks": causal_parity_masks[p],
                "ones": np.ones((P, 1), dtype=np.float32),
                "y": None,  # output
            }
        )
    for m in in_maps:
        del m["y"]
    return in_maps


def _parity_masks():
    """masks[p][kk, s, i] = NEG where key (128*s + kk) of the diagonal band
    is masked for local row i of parity p (global row = 2*i + p mod 512)."""
    out = []
    kk = np.arange(P)[:, None, None]
    s = np.arange(4)[None, :, None]
    i = np.arange(QW)[None, None, :]
    for p in range(2):
        m = np.where(128 * s + kk > 2 * i + p, np.float32(NEG), np.float32(0.0))
        out.append(np.ascontiguousarray(m.astype(np.float32)))
    return out


def _numpy_fallback(query, key, value, attn_mask, out_w, out_b):
    q = query.reshape(B, S, E).astype(np.float64) * (float(E) ** -0.5)
    k = key.reshape(B, S, E).astype(np.float64)
    v = value.reshape(B, S, E).astype(np.float64)
    scores = np.einsum("bqe,bke->bqk", q, k)
    scores = np.where(attn_mask[None, :, :] == 0, -np.inf, scores)
    scores -= scores.max(axis=-1, keepdims=True)
    probs = np.exp(scores)
    probs /= probs.sum(axis=-1, keepdims=True)
    attn = np.einsum("bqk,bke->bqe", probs, v)
    return (attn @ out_w.T.astype(np.float64) + out_b.astype(np.float64)).astype(
        np.float32
    )


def kernel(query, key, value, qkv_proj, attn_mask, out_w, out_b):
    del qkv_proj
    mask = np.asarray(attn_mask)
    is_causal = bool(
        np.array_equal(mask, np.tril(np.ones((S, S), dtype=mask.dtype)))
    )
    is_full = bool((mask != 0).all())
    if not (is_causal or is_full):
        return _numpy_fallback(query, key, value, mask, out_w, out_b)

    query = np.asarray(query, dtype=np.float32)
    key = np.asarray(key, dtype=np.float32)
    value = np.asarray(value, dtype=np.float32)
    out_w = np.asarray(out_w, dtype=np.float32)
    out_b = np.asarray(out_b, dtype=np.float32)

    nc = _get_program(causal=is_causal)
    in_maps = _make_in_maps(query, key, value, out_w, _parity_masks())
    res = run_bass_kernel_spmd(nc, in_maps, list(range(NCORES)))

    out = np.empty((B, S, E), dtype=np.float32)
    for c in range(NCORES):
        b, p = divmod(c, 2)
        y_un = res.results[c]["y"]
        lsum = res.results[c]["lsum"].reshape(NT * QW, 1)
        out[b, p::2, :] = y_un / lsum + out_b[None, :]
    return out


if __name__ == "__main__":
    rng = np.random.default_rng(0)
    q = rng.standard_normal((B, S, H, D), dtype=np.float32)
    k = rng.standard_normal((B, S, H, D), dtype=np.float32)
    v = rng.standard_normal((B, S, H, D), dtype=np.float32)
    w = rng.standard_normal((E, E), dtype=np.float32) * (1.0 / 32)
    bb = rng.standard_normal((E,), dtype=np.float32) * (1.0 / 32)
    m = np.tril(np.ones((S, S), dtype=np.int32))
    y = kernel(
        query=q, key=k, value=v, qkv_proj=np.zeros(1, np.float32),
        attn_mask=m, out_w=w, out_b=bb,
    )
    ref = _numpy_fallback(q, k, v, m, w, bb)
    err = np.abs(y - ref)
    rel = err.max() / np.abs(ref).max()
    print("quick self-check: absmax rel err =", rel)
